# revision 32
# baseline (speedup 1.0000x reference)
"""Multi-head self-attention Trainium2 Bass kernel.

Problem: y = (softmax((x@Wq)(x@Wk)^T / sqrt(hd)) (x@Wv)) @ Wp + biases
with B=4, T=2048, C=1024, H=16, hd=64.

Sharding over 8 NeuronCores: (batch b in 0..3) x (head-group g in 0..1, 8
heads each) -- tensor-parallel over heads, data-parallel over batch.  Each
core computes the attention for its batch and head group plus the partial
output projection restricted to its head group's features; the host sums
the two head-group partials per batch (the row-parallel TP reduction) and
transposes back.

Per-core pipeline (T=2048, C=1024, Cg=512), built to keep the Activation
engine (exp: 33.5M elements ~ 255us busy) saturated, since it is the hard
roofline for this op:

  stage 1 (bf16): Q^T,K^T [feat,tok] and V [tok,feat] from x tiles.
  stage 2 per head-pair: scores^T chunks [key,q] in PSUM (contract hd=64,
    two heads on disjoint PE row groups), exp via ScalarE with fused
    scale=1/8 and bias=-4ln2 (keeps e^s inside fp8e4 range; softmax is
    invariant to the common factor) emitting fp8e4 E tiles laid out
    [128, 2, 512] (two key-chunks side by side), then a DoubleRow fp8
    matmul with stationary [V_h|ones] packed [128, 2, 128] contracts 256
    keys per pass -- half the PE time of bf16 -- accumulating U^T and the
    replicated softmax denominator in one PSUM tile.
  The whole (pair, qc, kcpair) iteration space is software-pipelined:
    AV matmuls lag one step behind exp, and the PE slack in each
    ACT-bound step is filled with "extra" work units (pair 0: remaining
    V chunks; pairs 0-2: the next pair's QK chunks; pair 3: early
    output-projection chunks), so ScalarE rarely waits on TensorE.
  stage 3: out^T = (Y @ w_p)^T + b_out, streamed out per chunk.
"""

import numpy as np
import ml_dtypes

import concourse.bass as bass
import concourse.bacc as bacc
import concourse.tile as tile
from concourse import mybir
from concourse.bass_utils import run_bass_kernel_spmd

N_CORES = 8
C = 1024           # embed dim
H = 16             # total heads
HD = 64            # head dim
HPC = 8            # heads per core
CG = HPC * HD      # 512: per-core q/k/v feature width

F32 = mybir.dt.float32
BF16 = mybir.dt.bfloat16
FP8 = mybir.dt.float8e4
NP_FP8 = ml_dtypes.float8_e4m3
NP_BF16 = ml_dtypes.bfloat16

# exp(s/8 - 4ln2): rescales E by 1/16 so e^s stays far from the fp8e4
# saturation point (240); softmax is invariant to the common factor.
EXP_BIAS = float(-4.0 * np.log(2.0))

DR = mybir.MatmulPerfMode.DoubleRow


def _body(tc, T, x_t, w_qk, b_qk, w_v, w_p, b_out, out_t):
    nc = tc.nc
    KC = C // 128            # contraction chunks over C (8)
    FC = 2 * CG // 128       # q||k feature chunks (8)
    TC4 = T // 512           # token chunks of 512 (4)
    TC1 = T // 128           # key chunks of 128 (16)
    KP = TC1 // 2            # key chunk PAIRS (8)
    OCC = C // 128           # output channel chunks (8)
    PCH = CG // 128          # proj contraction chunks (4)
    NPAIR = HPC // 2         # head pairs (4)
    QW = 512                 # q tile width in stage 2
    NQC = T // QW            # q chunks per pair (4)
    Exp = mybir.ActivationFunctionType.Exp
    Mult = mybir.AluOpType.mult

    with (
        tc.tile_pool(name="const", bufs=1) as constp,
        tc.tile_pool(name="persist", bufs=1) as pers,
        tc.tile_pool(name="wqk", bufs=2) as wqkp,
        tc.tile_pool(name="wv", bufs=1) as wvp,
        tc.tile_pool(name="wp", bufs=1) as wpp,
        tc.tile_pool(name="e", bufs=2) as ep,
        tc.tile_pool(name="rec", bufs=2) as recp,
        tc.tile_pool(name="outp", bufs=2) as outp,
        tc.tile_pool(name="ps", bufs=1, space="PSUM") as psp,
    ):
        bqk_sb = constp.tile([128, FC], F32, tag="bqk")
        nc.sync.dma_start(bqk_sb[:], b_qk[:])
        bout_sb = constp.tile([128, OCC], F32, tag="bout")

        # x tiles [128, T] bf16, one per contraction chunk.  DMA'd in
        # token-quarters (all tiles' quarter q before quarter q+1) so the
        # first QK chunks (which only need tokens 0:512 of every tile) can
        # start ~3.5us in instead of waiting for the full 4MB of x.
        xt = [pers.tile([128, T], BF16, tag=f"xt{i}", name=f"xt{i}")
              for i in range(KC)]

        # stationary V packs for the AV matmul: per key-chunk kc, [128, 8*128]
        # bf16 where cols h*128+(0:64) hold V_h for key 128*kc + p and cols
        # h*128+(64:128) hold ones (the softmax-denominator rows, replicated
        # so one M=128 matmul yields U^T and the denominator).  memset to 1.0
        # once; V copies fill the V halves.
        v2 = [pers.tile([128, 8 * 128], BF16, tag=f"v2_{i}", name=f"v2_{i}")
              for i in range(TC1)]
        for i in range(TC1):
            nc.gpsimd.memset(v2[i][:], 1.0)

        qkt = [pers.tile([128, T], BF16, tag=f"qkt{i}", name=f"qkt{i}")
               for i in range(FC)]
        yt = [pers.tile([128, T], BF16, tag=f"yt{i}", name=f"yt{i}")
              for i in range(PCH)]

        # consolidated weight tiles (host-permuted so each is ONE dma with
        # >=2KB contiguous per-partition runs -- DMA dispatch is ~650ns per
        # instruction regardless of size, so fewer+bigger wins)
        wv_all = wvp.tile([128, KC * CG], BF16, tag="wv", name="wv_all")
        wp_all = wpp.tile([128, PCH * C], BF16, tag="wp", name="wp_all")
        wv = [wv_all[:, kc * CG:(kc + 1) * CG] for kc in range(KC)]
        wp = [wp_all[:, fcp * C:(fcp + 1) * C] for fcp in range(PCH)]

        # ---------- helper emitters ----------

        def dma_wqk(pair):
            """DMA the two w_qk feature chunks (K first: fc=4+pair, then
            Q: fc=pair) -- host-permuted layout, one DMA per fc.  The tile
            holds all KC stationary blocks: [p, kc, col]."""
            tiles = {}
            for fc in (FC // 2 + pair, pair):
                t = wqkp.tile([128, KC * 128], BF16, tag=f"wqk_{fc % 2}",
                              name=f"wqk_{fc}")
                nc.sync.dma_start(t[:], w_qk[fc * 128:(fc + 1) * 128, :])
                v = t.rearrange("p (kc c) -> p kc c", c=128)
                tiles[fc] = [v[:, kc, :] for kc in range(KC)]
            return tiles

        # "extra" PE work is emitted in ~2-matmul units (~430 ns) so a unit
        # fits the per-step PE slack of the ACT-bound steady state.

        def qk_units(wts, fc, t4, slot):
            """One QK output chunk qkt[fc][:, t4*512:] as KC//2 units."""
            st = {}

            def unit(u):
                if u == 0:
                    st["ps"] = psp.tile([128, 512], F32, tag=slot, bufs=1,
                                        name=f"qkps_{fc}_{t4}")
                ps = st["ps"]
                for kc in (2 * u, 2 * u + 1):
                    nc.tensor.matmul(
                        ps[:], wts[kc][:], xt[kc][:, t4 * 512:(t4 + 1) * 512],
                        start=(kc == 0), stop=(kc == KC - 1))
                if u == KC // 2 - 1:
                    nc.vector.tensor_scalar_add(
                        qkt[fc][:, t4 * 512:(t4 + 1) * 512], ps[:],
                        bqk_sb[:, fc:fc + 1])

            return [lambda u=u: unit(u) for u in range(KC // 2)]

        def v_units(tokc, slot):
            """One V token chunk -> fp8 copy into the v2 pack, as units."""
            st = {}

            def unit(u):
                if u == 0:
                    st["ps"] = psp.tile([128, CG], F32, tag=slot, bufs=1,
                                        name=f"vps_{tokc}")
                ps = st["ps"]
                for kc in (2 * u, 2 * u + 1):
                    nc.tensor.matmul(
                        ps[:], xt[kc][:, tokc * 128:(tokc + 1) * 128],
                        wv[kc][:],
                        start=(kc == 0), stop=(kc == KC - 1))
                if u == KC // 2 - 1:
                    dst = v2[tokc].rearrange("p (h c) -> p h c", c=128)
                    nc.vector.tensor_copy(
                        dst[:, :, 0:HD],
                        ps.rearrange("p (h c) -> p h c", c=HD))

            return [lambda u=u: unit(u) for u in range(KC // 2)]

        def proj_units(occ, t4, slot):
            """One projection output chunk [128, 512] -> DMA out, as units."""
            st = {}

            def unit(u):
                if u == 0:
                    st["ps"] = psp.tile([128, 512], F32, tag=slot, bufs=1,
                                        name=f"pps_{occ}_{t4}")
                ps = st["ps"]
                for fcp in (2 * u, 2 * u + 1):
                    nc.tensor.matmul(
                        ps[:], wp[fcp][:, occ * 128:(occ + 1) * 128],
                        yt[fcp][:, t4 * 512:(t4 + 1) * 512],
                        start=(fcp == 0), stop=(fcp == PCH - 1))
                if u == PCH // 2 - 1:
                    osb = outp.tile([128, 512], F32, tag="osb",
                                    name=f"osb_{occ}_{t4}")
                    nc.vector.tensor_scalar_add(osb[:], ps[:],
                                                bout_sb[:, occ:occ + 1])
                    nc.sync.dma_start(
                        out_t[occ * 128:(occ + 1) * 128,
                              t4 * 512:(t4 + 1) * 512],
                        osb[:])

            return [lambda u=u: unit(u) for u in range(PCH // 2)]

        def v_chunk(tokc, slot):
            for u in v_units(tokc, slot):
                u()

        def proj_chunk(occ, t4, slot):
            for u in proj_units(occ, t4, slot):
                u()

        def emit_av(pend):
            """AV matmuls for one step (two key chunks), one head s."""
            s, i, et, u, pair = pend
            h = 2 * pair + s
            for half in (0, 1):
                kc = 2 * i + half
                nc.tensor.matmul(
                    u[:], v2[kc][:, h * 128:(h + 1) * 128],
                    et[:, half * QW:(half + 1) * QW],
                    start=(i == 0 and half == 0),
                    stop=(i == KP - 1 and half == 1))

        def normalize(pair, qc, ups):
            q0 = qc * QW
            for s in (0, 1):
                po = s * 64
                u = ups[s]
                rec = recp.tile([64, QW], F32, tag=f"rec{s}",
                                name=f"rec{s}_{pair}_{qc}")
                nc.vector.reciprocal(rec[:], u[64:128, :])
                nc.vector.tensor_tensor(
                    yt[pair][po:po + 64, q0:q0 + QW],
                    u[0:64, :], rec[:], op=Mult)

        # ---------- DMA emission order (startup-latency critical) ----------
        # pair-0 K/Q weights, then x tiles, then V/proj weights, biases.
        wqk_tiles = {0: dma_wqk(0)}
        for i in range(KC):
            nc.sync.dma_start(xt[i][:], x_t[i * 128:(i + 1) * 128, :])
        nc.sync.dma_start(wv_all[:], w_v[:])
        nc.sync.dma_start(wp_all[:], w_p[:])
        nc.sync.dma_start(bout_sb[:], b_out[:])

        # ---------- prologue: pair-0 K (all t4) + Q-t0, then V chunks ----
        N_PRO_V = min(TC1, 13)
        pro_slots = ["aux0", "aux1", "ups0", "ups1"]
        pro_chunks = [(FC // 2, t4) for t4 in range(TC4)] + [(0, 0)]
        for n, (fc, t4) in enumerate(pro_chunks):
            for u in qk_units(wqk_tiles[0][fc], fc, t4,
                              slot=pro_slots[n % 4]):
                u()
        for tokc in range(N_PRO_V):
            v_chunk(tokc, slot=pro_slots[tokc % 4])

        # ---------- per-block extra PE work (unit-granular) ----------
        def queue_qk_pair(pair_next, extra_q):
            wqk_tiles[pair_next] = dma_wqk(pair_next)
            for j, fc in enumerate(
                    [pair_next] * TC4 + [FC // 2 + pair_next] * TC4):
                t4 = j % TC4
                extra_q.extend(
                    qk_units(wqk_tiles[pair_next][fc], fc, t4,
                             slot=f"aux{j % 2}"))

        def block_extra(pair, qc, extra_q):
            if pair == 0:
                if qc == 0:
                    for tokc in range(N_PRO_V, TC1):
                        extra_q.extend(v_units(tokc, slot=f"aux{tokc % 2}"))
                    # pair-0 Q chunks for the later q-blocks (Q-t_qc is
                    # needed at block (0, qc); t0 was in the prologue)
                    for j, t4 in enumerate(range(1, TC4)):
                        extra_q.extend(
                            qk_units(wqk_tiles[0][0], 0, t4,
                                     slot=f"aux{j % 2}"))
                if qc == min(1, NQC - 1) and NPAIR > 1:
                    queue_qk_pair(1, extra_q)
            elif pair < NPAIR - 1:
                if qc == 0:
                    queue_qk_pair(pair + 1, extra_q)
            else:
                # early proj chunks: yt for t4 is final once block qc=t4's
                # lagged AVs and normalize have been emitted, which happens
                # at step (3, t4+1, 0) before this block's pops run.
                t4 = qc - 1
                if 0 <= t4 < TC4:
                    for occ in range(OCC):
                        extra_q.extend(
                            proj_units(occ, t4, slot=f"aux{occ % 2}"))

        def pop_budget(pair, qc):
            if pair == 0:
                return 3 if qc == 0 else 2
            if pair == NPAIR - 1:
                return 2
            return 1

        # ---------- main software-pipelined loop ----------
        steps = [(pair, qc, i)
                 for pair in range(NPAIR)
                 for qc in range(NQC)
                 for i in range(KP)]

        extra_q = []
        pend = {0: None, 1: None}    # per-s pending AV from previous step
        done_block = None            # (pair, qc, ups) awaiting normalize
        ups = None

        for (pair, qc, i) in steps:
            if i == 0 and qc == 0 and pair > 0:
                # emission-order safety: everything queued for earlier pairs
                # (e.g. this pair's QK chunks) must be emitted before this
                # pair's scores read the tiles it writes.
                while extra_q:
                    extra_q.pop(0)()
            if i == 0:
                block_extra(pair, qc, extra_q)
                ups = {s: psp.tile([128, QW], F32, tag=f"ups{s}", bufs=1,
                                   name=f"ups{s}_{pair}_{qc}")
                       for s in (0, 1)}

            qt, kt = qkt[pair], qkt[FC // 2 + pair]
            q0 = qc * QW

            for s in (0, 1):
                # lagged AV from the previous step for this s
                if pend[s] is not None:
                    emit_av(pend[s])
                    pend[s] = None
                # scores: sps_s holds key-chunks 2i | 2i+1 side by side
                po = s * 64
                sps = psp.tile([128, 2 * QW], F32, tag=f"sps{s}", bufs=1,
                               name=f"sps{s}_{pair}_{qc}_{i}")
                for half in (0, 1):
                    kc = 2 * i + half
                    nc.tensor.matmul(
                        sps[:, half * QW:(half + 1) * QW],
                        kt[po:po + 64, kc * 128:(kc + 1) * 128],
                        qt[po:po + 64, q0:q0 + QW],
                        start=True, stop=True)
                # exp -> bf16 E tile (ACT queue)
                et = ep.tile([128, 2 * QW], BF16, tag=f"et{s}",
                             name=f"et{s}_{pair}_{qc}_{i}")
                nc.scalar.activation(et[:], sps[:], Exp, scale=0.125)
                pend[s] = (s, i, et, ups[s], pair)

            # normalize the previous block once its last AVs are emitted
            # (they were emitted above, during this step s-loop, iff i == 0)
            if i == 0 and done_block is not None:
                normalize(*done_block)
                done_block = None
            if i == KP - 1:
                done_block = (pair, qc, ups)

            # a few units of extra PE work per step (~430 ns each)
            for _ in range(pop_budget(pair, qc)):
                if extra_q:
                    extra_q.pop(0)()

        # epilogue: last step's AVs, last block's normalize, leftovers
        for s in (0, 1):
            if pend[s] is not None:
                emit_av(pend[s])
                pend[s] = None
        if done_block is not None:
            normalize(*done_block)
        for th in extra_q:
            th()
        # remaining proj chunks: t4 = NQC-1 .. TC4-1 (inline covered 0..NQC-2)
        # All four PSUM slots are free now -- rotate them for a deeper
        # chunk pipeline in this PE-bound tail.
        n = 0
        for t4 in range(NQC - 1, TC4):
            for occ in range(OCC):
                proj_chunk(occ, t4, slot=pro_slots[n % 4])
                n += 1


def build_nc(T=2048):
    FC = 2 * CG // 128
    OCC = C // 128
    nc = bacc.Bacc("TRN2", target_bir_lowering=False, debug=False,
                   num_devices=N_CORES)
    KC = C // 128
    PCH = CG // 128
    x_t = nc.dram_tensor("x_t", [C, T], BF16, kind="ExternalInput")
    # host-permuted: w_qk[fc*128+p, kc*128+c] = W[kc*128+p, fc*128+c]
    w_qk = nc.dram_tensor("w_qk", [FC * 128, KC * 128], BF16,
                          kind="ExternalInput")
    b_qk = nc.dram_tensor("b_qk", [128, FC], F32, kind="ExternalInput")
    # host-permuted: w_v[p, kc*CG+c] = Wv[kc*128+p, c]
    w_v = nc.dram_tensor("w_v", [128, KC * CG], BF16, kind="ExternalInput")
    # host-permuted: w_p[p, fcp*C+c] = Wp[fcp*128+p, c]
    w_p = nc.dram_tensor("w_p", [128, PCH * C], BF16, kind="ExternalInput")
    b_out = nc.dram_tensor("b_out", [128, OCC], F32, kind="ExternalInput")
    out_t = nc.dram_tensor("out_t", [C, T], F32, kind="ExternalOutput")
    with tile.TileContext(nc) as tc:
        _body(tc, T, x_t.ap(), w_qk.ap(), b_qk.ap(), w_v.ap(),
              w_p.ap(), b_out.ap(), out_t.ap())
    nc.compile()
    return nc


def shard_inputs(sequences, w_attn, b_attn, w_proj, b_proj):
    """Build the 8 per-core input maps. Core index = b*2 + g."""
    sequences = np.asarray(sequences, dtype=np.float32)
    w_attn = np.asarray(w_attn, dtype=np.float32)
    b_attn = np.asarray(b_attn, dtype=np.float32)
    w_proj = np.asarray(w_proj, dtype=np.float32)
    b_proj = np.asarray(b_proj, dtype=np.float32)
    B = sequences.shape[0]
    in_maps = []
    for b in range(B):
        xt_b = np.ascontiguousarray(sequences[b].T).astype(NP_BF16)
        for g in range(2):
            qs = slice(g * CG, (g + 1) * CG)
            ks = slice(C + g * CG, C + (g + 1) * CG)
            vs = slice(2 * C + g * CG, 2 * C + (g + 1) * CG)
            KC, FC, PCH = C // 128, 2 * CG // 128, CG // 128
            wqk_cat = np.concatenate([w_attn[:, qs], w_attn[:, ks]], axis=1)
            wqk_perm = (wqk_cat.reshape(KC, 128, FC, 128)
                        .transpose(2, 1, 0, 3).reshape(FC * 128, KC * 128))
            wv_perm = (w_attn[:, vs].reshape(KC, 128, CG)
                       .transpose(1, 0, 2).reshape(128, KC * CG))
            wp_perm = (w_proj[g * CG:(g + 1) * CG, :].reshape(PCH, 128, C)
                       .transpose(1, 0, 2).reshape(128, PCH * C))
            in_maps.append({
                "x_t": xt_b,
                "w_qk": np.ascontiguousarray(wqk_perm).astype(NP_BF16),
                "b_qk": np.ascontiguousarray(
                    np.concatenate([b_attn[qs], b_attn[ks]])
                    .reshape(8, 128).T),
                "w_v": np.ascontiguousarray(wv_perm).astype(NP_BF16),
                "w_p": np.ascontiguousarray(wp_perm).astype(NP_BF16),
                # softmax rows sum to 1, so the v-bias folds into the output
                # bias: y_g = attn@(x@w_v) @ w_p + (b_v@w_p [+ b_proj on g0])
                "b_out": np.ascontiguousarray(
                    (b_attn[vs] @ w_proj[g * CG:(g + 1) * CG, :]
                     + (b_proj if g == 0 else 0.0))
                    .astype(np.float32).reshape(8, 128).T),
            })
    return in_maps


def unshard_outputs(outs, B, T):
    """outs: list of 8 [C, T] partials, core index = b*2+g."""
    y = np.empty((B, T, C), np.float32)
    for b in range(B):
        y[b] = (outs[2 * b] + outs[2 * b + 1]).T
    return y


_NC_CACHE = {}


def kernel(sequences, w_attn, b_attn, w_proj, b_proj):
    sequences = np.asarray(sequences, dtype=np.float32)
    B, T, _ = sequences.shape
    in_maps = shard_inputs(sequences, w_attn, b_attn, w_proj, b_proj)
    if T not in _NC_CACHE:
        _NC_CACHE[T] = build_nc(T)
    nc = _NC_CACHE[T]
    res = run_bass_kernel_spmd(nc, in_maps, list(range(N_CORES)))
    outs = [res.results[i]["out_t"] for i in range(N_CORES)]
    return unshard_outputs(outs, B, T)


if __name__ == "__main__":
    rng = np.random.default_rng(0)
    B, T = 4, 2048
    seq = rng.standard_normal((B, T, C), dtype=np.float32)
    wa = rng.standard_normal((C, 3 * C), dtype=np.float32) / np.sqrt(C)
    ba = np.zeros(3 * C, np.float32)
    wp = rng.standard_normal((C, C), dtype=np.float32) / np.sqrt(C)
    bp = np.zeros(C, np.float32)
    y = kernel(seq, wa, ba, wp, bp)
    print(y.shape, y.dtype)


# revision 33
# speedup vs baseline: 3.1479x; 3.1479x over previous
"""Multi-head self-attention Trainium2 Bass kernel.

Problem: y = (softmax((x@Wq)(x@Wk)^T / sqrt(hd)) (x@Wv)) @ Wp + biases
with B=4, T=2048, C=1024, H=16, hd=64.

Sharding over 8 NeuronCores: (batch b in 0..3) x (head-group g in 0..1, 8
heads each) -- tensor-parallel over heads, data-parallel over batch.  Each
core computes the attention for its batch and head group plus the partial
output projection restricted to its head group's features; the host sums
the two head-group partials per batch (the row-parallel TP reduction) and
transposes back.

Per-core pipeline (T=2048, C=1024, Cg=512), built to keep the Activation
engine (exp: 33.5M elements ~ 255us busy) saturated, since it is the hard
roofline for this op:

  stage 1 (bf16): Q^T,K^T [feat,tok] and V [tok,feat] from x tiles.
  stage 2 per head-pair: scores^T chunks [key,q] in PSUM (contract hd=64,
    two heads on disjoint PE row groups), exp via ScalarE with fused
    scale=1/8 emitting bf16 E tiles [128, 1024] (two key-chunks side by
    side -- 1024-element activations amortize the ~185ns ACT access
    overhead), then AV matmuls with stationary [V_h|ones] so one M=128
    pass accumulates U^T and the replicated softmax denominator (the
    ones columns are free: matmul cost is moving rows only).
  The whole (pair, qc, kcpair) iteration space is software-pipelined:
    AV matmuls lag one step behind exp, and the PE slack in each step is
    filled with "extra" ~2-matmul work units (pair 0: remaining V chunks
    and later Q chunks; pairs 0-2: the next pair's QK chunks; pair 3:
    early output-projection chunks), so neither PE nor ScalarE waits.
  stage 3: out^T = (Y @ w_p)^T + b_out, streamed out per chunk.

  All weights are host-permuted so each loads with a single large DMA
  (DMA dispatch is ~650ns per instruction on the sync queue regardless
  of size, so few+large transfers win).
"""

import numpy as np
import ml_dtypes

import concourse.bass as bass
import concourse.bacc as bacc
import concourse.tile as tile
from concourse import mybir
from concourse.bass_utils import run_bass_kernel_spmd

N_CORES = 8
C = 1024           # embed dim
H = 16             # total heads
HD = 64            # head dim
HPC = 8            # heads per core
CG = HPC * HD      # 512: per-core q/k/v feature width

F32 = mybir.dt.float32
BF16 = mybir.dt.bfloat16
NP_BF16 = ml_dtypes.bfloat16


def _body(tc, T, x_t, w_qk, b_qk, w_v, w_p, b_out, out_t):
    nc = tc.nc
    KC = C // 128            # contraction chunks over C (8)
    FC = 2 * CG // 128       # q||k feature chunks (8)
    TC4 = T // 512           # token chunks of 512 (4)
    TC1 = T // 128           # key chunks of 128 (16)
    KP = TC1 // 2            # key chunk PAIRS (8)
    OCC = C // 128           # output channel chunks (8)
    PCH = CG // 128          # proj contraction chunks (4)
    NPAIR = HPC // 2         # head pairs (4)
    QW = 512                 # q tile width in stage 2
    NQC = T // QW            # q chunks per pair (4)
    Exp = mybir.ActivationFunctionType.Exp
    Mult = mybir.AluOpType.mult

    with (
        tc.tile_pool(name="const", bufs=1) as constp,
        tc.tile_pool(name="persist", bufs=1) as pers,
        tc.tile_pool(name="wqk", bufs=2) as wqkp,
        tc.tile_pool(name="wv", bufs=1) as wvp,
        tc.tile_pool(name="wp", bufs=1) as wpp,
        tc.tile_pool(name="e", bufs=2) as ep,
        tc.tile_pool(name="rec", bufs=2) as recp,
        tc.tile_pool(name="outp", bufs=2) as outp,
        tc.tile_pool(name="ps", bufs=1, space="PSUM") as psp,
    ):
        bqk_sb = constp.tile([128, FC], F32, tag="bqk")
        nc.sync.dma_start(bqk_sb[:], b_qk[:])
        bout_sb = constp.tile([128, OCC], F32, tag="bout")

        # x tiles [128, T] bf16, one per contraction chunk.  DMA'd in
        # token-quarters (all tiles' quarter q before quarter q+1) so the
        # first QK chunks (which only need tokens 0:512 of every tile) can
        # start ~3.5us in instead of waiting for the full 4MB of x.
        xt = [pers.tile([128, T], BF16, tag=f"xt{i}", name=f"xt{i}")
              for i in range(KC)]

        # stationary V packs for the AV matmul: per key-chunk kc, [128, 8*128]
        # bf16 where cols h*128+(0:64) hold V_h for key 128*kc + p and cols
        # h*128+(64:128) hold ones (the softmax-denominator rows, replicated
        # so one M=128 matmul yields U^T and the denominator).  memset to 1.0
        # once; V copies fill the V halves.
        v2 = [pers.tile([128, 8 * 128], BF16, tag=f"v2_{i}", name=f"v2_{i}")
              for i in range(TC1)]
        for i in range(TC1):
            nc.gpsimd.memset(v2[i][:], 1.0)

        qkt = [pers.tile([128, T], BF16, tag=f"qkt{i}", name=f"qkt{i}")
               for i in range(FC)]
        yt = [pers.tile([128, T], BF16, tag=f"yt{i}", name=f"yt{i}")
              for i in range(PCH)]

        # consolidated weight tiles (host-permuted so each is ONE dma with
        # >=2KB contiguous per-partition runs -- DMA dispatch is ~650ns per
        # instruction regardless of size, so fewer+bigger wins)
        wv_all = wvp.tile([128, KC * CG], BF16, tag="wv", name="wv_all")
        wp_all = wpp.tile([128, PCH * C], BF16, tag="wp", name="wp_all")
        wv = [wv_all[:, kc * CG:(kc + 1) * CG] for kc in range(KC)]
        wp = [wp_all[:, fcp * C:(fcp + 1) * C] for fcp in range(PCH)]

        # ---------- helper emitters ----------

        def dma_wqk(pair):
            """DMA the two w_qk feature chunks (K first: fc=4+pair, then
            Q: fc=pair) -- host-permuted layout, one DMA per fc.  The tile
            holds all KC stationary blocks: [p, kc, col]."""
            tiles = {}
            for fc in (FC // 2 + pair, pair):
                t = wqkp.tile([128, KC * 128], BF16, tag=f"wqk_{fc % 2}",
                              name=f"wqk_{fc}")
                nc.sync.dma_start(t[:], w_qk[fc * 128:(fc + 1) * 128, :])
                v = t.rearrange("p (kc c) -> p kc c", c=128)
                tiles[fc] = [v[:, kc, :] for kc in range(KC)]
            return tiles

        # "extra" PE work is emitted in ~2-matmul units (~430 ns) so a unit
        # fits the per-step PE slack of the ACT-bound steady state.

        def qk_units(wts, fc, t4, slot):
            """One QK output chunk qkt[fc][:, t4*512:] as KC//2 units."""
            st = {}

            def unit(u):
                if u == 0:
                    st["ps"] = psp.tile([128, 512], F32, tag=slot, bufs=1,
                                        name=f"qkps_{fc}_{t4}")
                ps = st["ps"]
                for kc in (2 * u, 2 * u + 1):
                    nc.tensor.matmul(
                        ps[:], wts[kc][:], xt[kc][:, t4 * 512:(t4 + 1) * 512],
                        start=(kc == 0), stop=(kc == KC - 1))
                if u == KC // 2 - 1:
                    nc.vector.tensor_scalar_add(
                        qkt[fc][:, t4 * 512:(t4 + 1) * 512], ps[:],
                        bqk_sb[:, fc:fc + 1])

            return [lambda u=u: unit(u) for u in range(KC // 2)]

        def v_units(tokc, slot):
            """One V token chunk -> fp8 copy into the v2 pack, as units."""
            st = {}

            def unit(u):
                if u == 0:
                    st["ps"] = psp.tile([128, CG], F32, tag=slot, bufs=1,
                                        name=f"vps_{tokc}")
                ps = st["ps"]
                for kc in (2 * u, 2 * u + 1):
                    nc.tensor.matmul(
                        ps[:], xt[kc][:, tokc * 128:(tokc + 1) * 128],
                        wv[kc][:],
                        start=(kc == 0), stop=(kc == KC - 1))
                if u == KC // 2 - 1:
                    dst = v2[tokc].rearrange("p (h c) -> p h c", c=128)
                    nc.vector.tensor_copy(
                        dst[:, :, 0:HD],
                        ps.rearrange("p (h c) -> p h c", c=HD))

            return [lambda u=u: unit(u) for u in range(KC // 2)]

        def proj_units(occ, t4, slot):
            """One projection output chunk [128, 512] -> DMA out, as units."""
            st = {}

            def unit(u):
                if u == 0:
                    st["ps"] = psp.tile([128, 512], F32, tag=slot, bufs=1,
                                        name=f"pps_{occ}_{t4}")
                ps = st["ps"]
                for fcp in (2 * u, 2 * u + 1):
                    nc.tensor.matmul(
                        ps[:], wp[fcp][:, occ * 128:(occ + 1) * 128],
                        yt[fcp][:, t4 * 512:(t4 + 1) * 512],
                        start=(fcp == 0), stop=(fcp == PCH - 1))
                if u == PCH // 2 - 1:
                    osb = outp.tile([128, 512], F32, tag="osb",
                                    name=f"osb_{occ}_{t4}")
                    nc.vector.tensor_scalar_add(osb[:], ps[:],
                                                bout_sb[:, occ:occ + 1])
                    nc.sync.dma_start(
                        out_t[occ * 128:(occ + 1) * 128,
                              t4 * 512:(t4 + 1) * 512],
                        osb[:])

            return [lambda u=u: unit(u) for u in range(PCH // 2)]

        def v_chunk(tokc, slot):
            for u in v_units(tokc, slot):
                u()

        def proj_chunk(occ, t4, slot):
            for u in proj_units(occ, t4, slot):
                u()

        def emit_av(pend):
            """AV matmuls for one step (two key chunks), one head s."""
            s, i, et, u, pair = pend
            h = 2 * pair + s
            for half in (0, 1):
                kc = 2 * i + half
                nc.tensor.matmul(
                    u[:], v2[kc][:, h * 128:(h + 1) * 128],
                    et[:, half * QW:(half + 1) * QW],
                    start=(i == 0 and half == 0),
                    stop=(i == KP - 1 and half == 1))

        def normalize(pair, qc, ups):
            q0 = qc * QW
            for s in (0, 1):
                po = s * 64
                u = ups[s]
                rec = recp.tile([64, QW], F32, tag=f"rec{s}",
                                name=f"rec{s}_{pair}_{qc}")
                nc.vector.reciprocal(rec[:], u[64:128, :])
                nc.vector.tensor_tensor(
                    yt[pair][po:po + 64, q0:q0 + QW],
                    u[0:64, :], rec[:], op=Mult)

        # ---------- DMA emission order (startup-latency critical) ----------
        # pair-0 K/Q weights, then x tiles, then V/proj weights, biases.
        wqk_tiles = {0: dma_wqk(0)}
        for i in range(KC):
            nc.sync.dma_start(xt[i][:], x_t[i * 128:(i + 1) * 128, :])
        nc.sync.dma_start(wv_all[:], w_v[:])
        nc.sync.dma_start(wp_all[:], w_p[:])
        nc.sync.dma_start(bout_sb[:], b_out[:])

        # ---------- prologue: pair-0 K (all t4) + Q-t0, then V chunks ----
        N_PRO_V = min(TC1, 13)
        pro_slots = ["aux0", "aux1", "ups0", "ups1"]
        pro_chunks = [(FC // 2, t4) for t4 in range(TC4)] + [(0, 0)]
        for n, (fc, t4) in enumerate(pro_chunks):
            for u in qk_units(wqk_tiles[0][fc], fc, t4,
                              slot=pro_slots[n % 4]):
                u()
        for tokc in range(N_PRO_V):
            v_chunk(tokc, slot=pro_slots[tokc % 4])

        # ---------- per-block extra PE work (unit-granular) ----------
        def queue_qk_pair(pair_next, extra_q):
            wqk_tiles[pair_next] = dma_wqk(pair_next)
            for j, fc in enumerate(
                    [pair_next] * TC4 + [FC // 2 + pair_next] * TC4):
                t4 = j % TC4
                extra_q.extend(
                    qk_units(wqk_tiles[pair_next][fc], fc, t4,
                             slot=f"aux{j % 2}"))

        def block_extra(pair, qc, extra_q):
            if pair == 0:
                if qc == 0:
                    for tokc in range(N_PRO_V, TC1):
                        extra_q.extend(v_units(tokc, slot=f"aux{tokc % 2}"))
                    # pair-0 Q chunks for the later q-blocks (Q-t_qc is
                    # needed at block (0, qc); t0 was in the prologue)
                    for j, t4 in enumerate(range(1, TC4)):
                        extra_q.extend(
                            qk_units(wqk_tiles[0][0], 0, t4,
                                     slot=f"aux{j % 2}"))
                if qc == min(1, NQC - 1) and NPAIR > 1:
                    queue_qk_pair(1, extra_q)
            elif pair < NPAIR - 1:
                if qc == 0:
                    queue_qk_pair(pair + 1, extra_q)
            else:
                # early proj chunks: yt for t4 is final once block qc=t4's
                # lagged AVs and normalize have been emitted, which happens
                # at step (3, t4+1, 0) before this block's pops run.
                t4 = qc - 1
                if 0 <= t4 < TC4:
                    for occ in range(OCC):
                        extra_q.extend(
                            proj_units(occ, t4, slot=f"aux{occ % 2}"))

        def pop_budget(pair, qc):
            if pair == 0:
                return 3 if qc == 0 else 2
            if pair == NPAIR - 1:
                return 2
            return 1

        # ---------- main software-pipelined loop ----------
        steps = [(pair, qc, i)
                 for pair in range(NPAIR)
                 for qc in range(NQC)
                 for i in range(KP)]

        extra_q = []
        pend = {0: None, 1: None}    # per-s pending AV from previous step
        done_block = None            # (pair, qc, ups) awaiting normalize
        ups = None

        for (pair, qc, i) in steps:
            if i == 0 and qc == 0 and pair > 0:
                # emission-order safety: everything queued for earlier pairs
                # (e.g. this pair's QK chunks) must be emitted before this
                # pair's scores read the tiles it writes.
                while extra_q:
                    extra_q.pop(0)()
            if i == 0:
                block_extra(pair, qc, extra_q)
                ups = {s: psp.tile([128, QW], F32, tag=f"ups{s}", bufs=1,
                                   name=f"ups{s}_{pair}_{qc}")
                       for s in (0, 1)}

            qt, kt = qkt[pair], qkt[FC // 2 + pair]
            q0 = qc * QW

            for s in (0, 1):
                # lagged AV from the previous step for this s
                if pend[s] is not None:
                    emit_av(pend[s])
                    pend[s] = None
                # scores: sps_s holds key-chunks 2i | 2i+1 side by side
                po = s * 64
                sps = psp.tile([128, 2 * QW], F32, tag=f"sps{s}", bufs=1,
                               name=f"sps{s}_{pair}_{qc}_{i}")
                for half in (0, 1):
                    kc = 2 * i + half
                    nc.tensor.matmul(
                        sps[:, half * QW:(half + 1) * QW],
                        kt[po:po + 64, kc * 128:(kc + 1) * 128],
                        qt[po:po + 64, q0:q0 + QW],
                        start=True, stop=True)
                # exp -> bf16 E tile (ACT queue)
                et = ep.tile([128, 2 * QW], BF16, tag=f"et{s}",
                             name=f"et{s}_{pair}_{qc}_{i}")
                nc.scalar.activation(et[:], sps[:], Exp, scale=0.125)
                pend[s] = (s, i, et, ups[s], pair)

            # normalize the previous block once its last AVs are emitted
            # (they were emitted above, during this step s-loop, iff i == 0)
            if i == 0 and done_block is not None:
                normalize(*done_block)
                done_block = None
            if i == KP - 1:
                done_block = (pair, qc, ups)

            # a few units of extra PE work per step (~430 ns each)
            for _ in range(pop_budget(pair, qc)):
                if extra_q:
                    extra_q.pop(0)()

        # epilogue: last step's AVs, last block's normalize, leftovers
        for s in (0, 1):
            if pend[s] is not None:
                emit_av(pend[s])
                pend[s] = None
        if done_block is not None:
            normalize(*done_block)
        for th in extra_q:
            th()
        # remaining proj chunks: t4 = NQC-1 .. TC4-1 (inline covered 0..NQC-2)
        # All four PSUM slots are free now -- rotate them for a deeper
        # chunk pipeline in this PE-bound tail.
        n = 0
        for t4 in range(NQC - 1, TC4):
            for occ in range(OCC):
                proj_chunk(occ, t4, slot=pro_slots[n % 4])
                n += 1


def build_nc(T=2048):
    FC = 2 * CG // 128
    OCC = C // 128
    nc = bacc.Bacc("TRN2", target_bir_lowering=False, debug=False,
                   num_devices=N_CORES)
    KC = C // 128
    PCH = CG // 128
    x_t = nc.dram_tensor("x_t", [C, T], BF16, kind="ExternalInput")
    # host-permuted: w_qk[fc*128+p, kc*128+c] = W[kc*128+p, fc*128+c]
    w_qk = nc.dram_tensor("w_qk", [FC * 128, KC * 128], BF16,
                          kind="ExternalInput")
    b_qk = nc.dram_tensor("b_qk", [128, FC], F32, kind="ExternalInput")
    # host-permuted: w_v[p, kc*CG+c] = Wv[kc*128+p, c]
    w_v = nc.dram_tensor("w_v", [128, KC * CG], BF16, kind="ExternalInput")
    # host-permuted: w_p[p, fcp*C+c] = Wp[fcp*128+p, c]
    w_p = nc.dram_tensor("w_p", [128, PCH * C], BF16, kind="ExternalInput")
    b_out = nc.dram_tensor("b_out", [128, OCC], F32, kind="ExternalInput")
    out_t = nc.dram_tensor("out_t", [C, T], F32, kind="ExternalOutput")
    with tile.TileContext(nc) as tc:
        _body(tc, T, x_t.ap(), w_qk.ap(), b_qk.ap(), w_v.ap(),
              w_p.ap(), b_out.ap(), out_t.ap())
    nc.compile()
    return nc


def shard_inputs(sequences, w_attn, b_attn, w_proj, b_proj):
    """Build the 8 per-core input maps. Core index = b*2 + g."""
    sequences = np.asarray(sequences, dtype=np.float32)
    w_attn = np.asarray(w_attn, dtype=np.float32)
    b_attn = np.asarray(b_attn, dtype=np.float32)
    w_proj = np.asarray(w_proj, dtype=np.float32)
    b_proj = np.asarray(b_proj, dtype=np.float32)
    B = sequences.shape[0]
    in_maps = []
    for b in range(B):
        xt_b = np.ascontiguousarray(sequences[b].T).astype(NP_BF16)
        for g in range(2):
            qs = slice(g * CG, (g + 1) * CG)
            ks = slice(C + g * CG, C + (g + 1) * CG)
            vs = slice(2 * C + g * CG, 2 * C + (g + 1) * CG)
            KC, FC, PCH = C // 128, 2 * CG // 128, CG // 128
            wqk_cat = np.concatenate([w_attn[:, qs], w_attn[:, ks]], axis=1)
            wqk_perm = (wqk_cat.reshape(KC, 128, FC, 128)
                        .transpose(2, 1, 0, 3).reshape(FC * 128, KC * 128))
            wv_perm = (w_attn[:, vs].reshape(KC, 128, CG)
                       .transpose(1, 0, 2).reshape(128, KC * CG))
            wp_perm = (w_proj[g * CG:(g + 1) * CG, :].reshape(PCH, 128, C)
                       .transpose(1, 0, 2).reshape(128, PCH * C))
            in_maps.append({
                "x_t": xt_b,
                "w_qk": np.ascontiguousarray(wqk_perm).astype(NP_BF16),
                "b_qk": np.ascontiguousarray(
                    np.concatenate([b_attn[qs], b_attn[ks]])
                    .reshape(8, 128).T),
                "w_v": np.ascontiguousarray(wv_perm).astype(NP_BF16),
                "w_p": np.ascontiguousarray(wp_perm).astype(NP_BF16),
                # softmax rows sum to 1, so the v-bias folds into the output
                # bias: y_g = attn@(x@w_v) @ w_p + (b_v@w_p [+ b_proj on g0])
                "b_out": np.ascontiguousarray(
                    (b_attn[vs] @ w_proj[g * CG:(g + 1) * CG, :]
                     + (b_proj if g == 0 else 0.0))
                    .astype(np.float32).reshape(8, 128).T),
            })
    return in_maps


def unshard_outputs(outs, B, T):
    """outs: list of 8 [C, T] partials, core index = b*2+g."""
    y = np.empty((B, T, C), np.float32)
    for b in range(B):
        y[b] = (outs[2 * b] + outs[2 * b + 1]).T
    return y


_NC_CACHE = {}


def kernel(sequences, w_attn, b_attn, w_proj, b_proj):
    sequences = np.asarray(sequences, dtype=np.float32)
    B, T, _ = sequences.shape
    in_maps = shard_inputs(sequences, w_attn, b_attn, w_proj, b_proj)
    if T not in _NC_CACHE:
        _NC_CACHE[T] = build_nc(T)
    nc = _NC_CACHE[T]
    res = run_bass_kernel_spmd(nc, in_maps, list(range(N_CORES)))
    outs = [res.results[i]["out_t"] for i in range(N_CORES)]
    return unshard_outputs(outs, B, T)


if __name__ == "__main__":
    rng = np.random.default_rng(0)
    B, T = 4, 2048
    seq = rng.standard_normal((B, T, C), dtype=np.float32)
    wa = rng.standard_normal((C, 3 * C), dtype=np.float32) / np.sqrt(C)
    ba = np.zeros(3 * C, np.float32)
    wp = rng.standard_normal((C, C), dtype=np.float32) / np.sqrt(C)
    bp = np.zeros(C, np.float32)
    y = kernel(seq, wa, ba, wp, bp)
    print(y.shape, y.dtype)


# revision 34
# speedup vs baseline: 3.3572x; 1.0665x over previous
"""Multi-head self-attention Trainium2 Bass kernel.

Problem: y = (softmax((x@Wq)(x@Wk)^T / sqrt(hd)) (x@Wv)) @ Wp + biases
with B=4, T=2048, C=1024, H=16, hd=64.

Sharding over 8 NeuronCores: (batch b in 0..3) x (head-group g in 0..1, 8
heads each) -- tensor-parallel over heads, data-parallel over batch.  Each
core computes the attention for its batch and head group plus the partial
output projection restricted to its head group's features; the host sums
the two head-group partials per batch (the row-parallel TP reduction) and
transposes back.

Per-core pipeline (T=2048, C=1024, Cg=512), built to keep the Activation
engine (exp: 33.5M elements ~ 255us busy) saturated, since it is the hard
roofline for this op:

  stage 1 (bf16): Q^T,K^T [feat,tok] and V [tok,feat] from x tiles.
  stage 2 per head-pair: scores^T chunks [key,q] in PSUM (contract hd=64,
    two heads on disjoint PE row groups), exp via ScalarE with fused
    scale=1/8 emitting bf16 E tiles [128, 1024] (two key-chunks side by
    side -- 1024-element activations amortize the ~185ns ACT access
    overhead), then AV matmuls with stationary [V_h|ones] so one M=128
    pass accumulates U^T and the replicated softmax denominator (the
    ones columns are free: matmul cost is moving rows only).
  The whole (pair, qc, kcpair) iteration space is software-pipelined:
    AV matmuls lag one step behind exp, and the PE slack in each step is
    filled with "extra" ~2-matmul work units (pair 0: remaining V chunks
    and later Q chunks; pairs 0-2: the next pair's QK chunks; pair 3:
    early output-projection chunks), so neither PE nor ScalarE waits.
  stage 3: out^T = (Y @ w_p)^T + b_out, streamed out per chunk.

  All weights are host-permuted so each loads with a single large DMA
  (DMA dispatch is ~650ns per instruction on the sync queue regardless
  of size, so few+large transfers win).
"""

import numpy as np
import ml_dtypes

import concourse.bass as bass
import concourse.bacc as bacc
import concourse.tile as tile
from concourse import mybir
from concourse.bass_utils import run_bass_kernel_spmd

N_CORES = 8
C = 1024           # embed dim
H = 16             # total heads
HD = 64            # head dim
HPC = 8            # heads per core
CG = HPC * HD      # 512: per-core q/k/v feature width

F32 = mybir.dt.float32
BF16 = mybir.dt.bfloat16
NP_BF16 = ml_dtypes.bfloat16


def _body(tc, T, x_t, w_qk, b_qk, w_v, w_p, b_out, out_t):
    nc = tc.nc
    KC = C // 128            # contraction chunks over C (8)
    FC = 2 * CG // 128       # q||k feature chunks (8)
    TC4 = T // 512           # token chunks of 512 (4)
    TC1 = T // 128           # key chunks of 128 (16)
    KP = TC1 // 2            # key chunk PAIRS (8)
    OCC = C // 128           # output channel chunks (8)
    PCH = CG // 128          # proj contraction chunks (4)
    NPAIR = HPC // 2         # head pairs (4)
    QW = 512                 # q tile width in stage 2
    NQC = T // QW            # q chunks per pair (4)
    Exp = mybir.ActivationFunctionType.Exp
    Mult = mybir.AluOpType.mult

    with (
        tc.tile_pool(name="const", bufs=1) as constp,
        tc.tile_pool(name="persist", bufs=1) as pers,
        tc.tile_pool(name="wqk", bufs=2) as wqkp,
        tc.tile_pool(name="wv", bufs=1) as wvp,
        tc.tile_pool(name="wp", bufs=1) as wpp,
        tc.tile_pool(name="e", bufs=2) as ep,
        tc.tile_pool(name="rec", bufs=2) as recp,
        tc.tile_pool(name="outp", bufs=2) as outp,
        tc.tile_pool(name="ps", bufs=1, space="PSUM") as psp,
    ):
        bqk_sb = constp.tile([128, FC], F32, tag="bqk")
        nc.sync.dma_start(bqk_sb[:], b_qk[:])
        bout_sb = constp.tile([128, OCC], F32, tag="bout")

        # x tiles [128, T] bf16, one per contraction chunk.  DMA'd in
        # token-quarters (all tiles' quarter q before quarter q+1) so the
        # first QK chunks (which only need tokens 0:512 of every tile) can
        # start ~3.5us in instead of waiting for the full 4MB of x.
        xt = [pers.tile([128, T], BF16, tag=f"xt{i}", name=f"xt{i}")
              for i in range(KC)]

        # stationary V packs for the AV matmul: per key-chunk kc, [128, 8*128]
        # bf16 where cols h*128+(0:64) hold V_h for key 128*kc + p and cols
        # h*128+(64:128) hold ones (the softmax-denominator rows, replicated
        # so one M=128 matmul yields U^T and the denominator).  memset to 1.0
        # once; V copies fill the V halves.
        v2 = [pers.tile([128, 8 * 128], BF16, tag=f"v2_{i}", name=f"v2_{i}")
              for i in range(TC1)]
        for i in range(TC1):
            nc.gpsimd.memset(v2[i][:], 1.0)

        qkt = [pers.tile([128, T], BF16, tag=f"qkt{i}", name=f"qkt{i}")
               for i in range(FC)]
        yt = [pers.tile([128, T], BF16, tag=f"yt{i}", name=f"yt{i}")
              for i in range(PCH)]

        # consolidated weight tiles (host-permuted so each is ONE dma with
        # >=2KB contiguous per-partition runs -- DMA dispatch is ~650ns per
        # instruction regardless of size, so fewer+bigger wins)
        wv_all = wvp.tile([128, KC * CG], BF16, tag="wv", name="wv_all")
        wp_all = wpp.tile([128, PCH * C], BF16, tag="wp", name="wp_all")
        wv = [wv_all[:, kc * CG:(kc + 1) * CG] for kc in range(KC)]
        wp = [wp_all[:, fcp * C:(fcp + 1) * C] for fcp in range(PCH)]

        # ---------- helper emitters ----------

        def dma_wqk(pair):
            """DMA the two w_qk feature chunks (K first: fc=4+pair, then
            Q: fc=pair) -- host-permuted layout, one DMA per fc.  The tile
            holds all KC stationary blocks: [p, kc, col]."""
            tiles = {}
            for fc in (FC // 2 + pair, pair):
                t = wqkp.tile([128, KC * 128], BF16, tag=f"wqk_{fc % 2}",
                              name=f"wqk_{fc}")
                nc.sync.dma_start(t[:], w_qk[fc * 128:(fc + 1) * 128, :])
                v = t.rearrange("p (kc c) -> p kc c", c=128)
                tiles[fc] = [v[:, kc, :] for kc in range(KC)]
            return tiles

        # "extra" PE work is emitted in ~2-matmul units (~430 ns) so a unit
        # fits the per-step PE slack of the ACT-bound steady state.

        def qk_units(wts, fc, t4, slot):
            """One QK output chunk qkt[fc][:, t4*512:] as KC//2 units."""
            st = {}

            def unit(u):
                if u == 0:
                    st["ps"] = psp.tile([128, 512], F32, tag=slot, bufs=1,
                                        name=f"qkps_{fc}_{t4}")
                ps = st["ps"]
                for kc in (2 * u, 2 * u + 1):
                    nc.tensor.matmul(
                        ps[:], wts[kc][:], xt[kc][:, t4 * 512:(t4 + 1) * 512],
                        start=(kc == 0), stop=(kc == KC - 1))
                if u == KC // 2 - 1:
                    nc.vector.tensor_scalar_add(
                        qkt[fc][:, t4 * 512:(t4 + 1) * 512], ps[:],
                        bqk_sb[:, fc:fc + 1])

            return [lambda u=u: unit(u) for u in range(KC // 2)]

        def v_units(tokc, slot):
            """One V token chunk -> fp8 copy into the v2 pack, as units."""
            st = {}

            def unit(u):
                if u == 0:
                    st["ps"] = psp.tile([128, CG], F32, tag=slot, bufs=1,
                                        name=f"vps_{tokc}")
                ps = st["ps"]
                for kc in (2 * u, 2 * u + 1):
                    nc.tensor.matmul(
                        ps[:], xt[kc][:, tokc * 128:(tokc + 1) * 128],
                        wv[kc][:],
                        start=(kc == 0), stop=(kc == KC - 1))
                if u == KC // 2 - 1:
                    dst = v2[tokc].rearrange("p (h c) -> p h c", c=128)
                    nc.vector.tensor_copy(
                        dst[:, :, 0:HD],
                        ps.rearrange("p (h c) -> p h c", c=HD))

            return [lambda u=u: unit(u) for u in range(KC // 2)]

        def proj_units(occ, t4, slot):
            """One projection output chunk [128, 512] -> DMA out, as units."""
            st = {}

            def unit(u):
                if u == 0:
                    st["ps"] = psp.tile([128, 512], F32, tag=slot, bufs=1,
                                        name=f"pps_{occ}_{t4}")
                ps = st["ps"]
                for fcp in (2 * u, 2 * u + 1):
                    nc.tensor.matmul(
                        ps[:], wp[fcp][:, occ * 128:(occ + 1) * 128],
                        yt[fcp][:, t4 * 512:(t4 + 1) * 512],
                        start=(fcp == 0), stop=(fcp == PCH - 1))
                if u == PCH // 2 - 1:
                    osb = outp.tile([128, 512], F32, tag="osb",
                                    name=f"osb_{occ}_{t4}")
                    nc.vector.tensor_scalar_add(osb[:], ps[:],
                                                bout_sb[:, occ:occ + 1])
                    nc.sync.dma_start(
                        out_t[occ * 128:(occ + 1) * 128,
                              t4 * 512:(t4 + 1) * 512],
                        osb[:])

            return [lambda u=u: unit(u) for u in range(PCH // 2)]

        def v_chunk(tokc, slot):
            for u in v_units(tokc, slot):
                u()

        def proj_chunk(occ, t4, slot):
            for u in proj_units(occ, t4, slot):
                u()

        def emit_av(pend):
            """AV matmuls for one step (two key chunks), one head s."""
            s, i, et, u, pair = pend
            h = 2 * pair + s
            for half in (0, 1):
                kc = 2 * i + half
                nc.tensor.matmul(
                    u[:], v2[kc][:, h * 128:(h + 1) * 128],
                    et[:, half * QW:(half + 1) * QW],
                    start=(i == 0 and half == 0),
                    stop=(i == KP - 1 and half == 1))

        def normalize(pair, qc, ups):
            q0 = qc * QW
            for s in (0, 1):
                po = s * 64
                u = ups[s]
                rec = recp.tile([64, QW], F32, tag=f"rec{s}",
                                name=f"rec{s}_{pair}_{qc}")
                nc.vector.reciprocal(rec[:], u[64:128, :])
                nc.vector.tensor_tensor(
                    yt[pair][po:po + 64, q0:q0 + QW],
                    u[0:64, :], rec[:], op=Mult)

        # ---------- DMA emission order (startup-latency critical) ----------
        # pair-0 K/Q weights, then x tiles, then V/proj weights, biases.
        wqk_tiles = {0: dma_wqk(0)}
        for i in range(KC):
            nc.sync.dma_start(xt[i][:], x_t[i * 128:(i + 1) * 128, :])
        nc.sync.dma_start(wv_all[:], w_v[:])
        nc.sync.dma_start(wp_all[:], w_p[:])
        nc.sync.dma_start(bout_sb[:], b_out[:])

        # ---------- prologue: pair-0 K (all t4) + Q-t0, then V chunks ----
        N_PRO_V = min(TC1, 11)
        pro_slots = ["aux0", "aux1", "ups0", "ups1"]
        pro_chunks = [(FC // 2, t4) for t4 in range(TC4)] + [(0, 0)]
        for n, (fc, t4) in enumerate(pro_chunks):
            for u in qk_units(wqk_tiles[0][fc], fc, t4,
                              slot=pro_slots[n % 4]):
                u()
        for tokc in range(N_PRO_V):
            v_chunk(tokc, slot=pro_slots[tokc % 4])

        # ---------- per-block extra PE work (unit-granular) ----------
        def queue_qk_pair(pair_next, extra_q):
            wqk_tiles[pair_next] = dma_wqk(pair_next)
            for j, fc in enumerate(
                    [pair_next] * TC4 + [FC // 2 + pair_next] * TC4):
                t4 = j % TC4
                extra_q.extend(
                    qk_units(wqk_tiles[pair_next][fc], fc, t4,
                             slot=f"aux{j % 2}"))

        def block_extra(pair, qc, extra_q):
            if pair == 0:
                if qc == 0:
                    for tokc in range(N_PRO_V, TC1):
                        extra_q.extend(v_units(tokc, slot=f"aux{tokc % 2}"))
                    # pair-0 Q chunks for the later q-blocks (Q-t_qc is
                    # needed at block (0, qc); t0 was in the prologue)
                    for j, t4 in enumerate(range(1, TC4)):
                        extra_q.extend(
                            qk_units(wqk_tiles[0][0], 0, t4,
                                     slot=f"aux{j % 2}"))
                if qc == min(1, NQC - 1) and NPAIR > 1:
                    queue_qk_pair(1, extra_q)
            elif pair < NPAIR - 1:
                if qc == 0:
                    queue_qk_pair(pair + 1, extra_q)
            else:
                # early proj chunks: yt for t4 is final once block qc=t4's
                # lagged AVs and normalize have been emitted, which happens
                # at step (3, t4+1, 0) before this block's pops run.
                t4 = qc - 1
                if 0 <= t4 < TC4:
                    for occ in range(OCC):
                        extra_q.extend(
                            proj_units(occ, t4, slot=f"aux{occ % 2}"))

        def pop_budget(pair, qc):
            if pair == 0:
                return 3 if qc == 0 else 2
            if pair == NPAIR - 1:
                return 2
            return 1

        # ---------- main software-pipelined loop ----------
        steps = [(pair, qc, i)
                 for pair in range(NPAIR)
                 for qc in range(NQC)
                 for i in range(KP)]

        extra_q = []
        pend = {0: None, 1: None}    # per-s pending AV from previous step
        done_block = None            # (pair, qc, ups) awaiting normalize
        ups = None

        for (pair, qc, i) in steps:
            if i == 0 and qc == 0 and pair > 0:
                # emission-order safety: everything queued for earlier pairs
                # (e.g. this pair's QK chunks) must be emitted before this
                # pair's scores read the tiles it writes.
                while extra_q:
                    extra_q.pop(0)()
            if i == 0:
                block_extra(pair, qc, extra_q)
                ups = {s: psp.tile([128, QW], F32, tag=f"ups{s}", bufs=1,
                                   name=f"ups{s}_{pair}_{qc}")
                       for s in (0, 1)}

            qt, kt = qkt[pair], qkt[FC // 2 + pair]
            q0 = qc * QW

            for s in (0, 1):
                # lagged AV from the previous step for this s
                if pend[s] is not None:
                    emit_av(pend[s])
                    pend[s] = None
                # scores: sps_s holds key-chunks 2i | 2i+1 side by side
                po = s * 64
                sps = psp.tile([128, 2 * QW], F32, tag=f"sps{s}", bufs=1,
                               name=f"sps{s}_{pair}_{qc}_{i}")
                for half in (0, 1):
                    kc = 2 * i + half
                    nc.tensor.matmul(
                        sps[:, half * QW:(half + 1) * QW],
                        kt[po:po + 64, kc * 128:(kc + 1) * 128],
                        qt[po:po + 64, q0:q0 + QW],
                        start=True, stop=True)
                # exp -> bf16 E tile (ACT queue)
                et = ep.tile([128, 2 * QW], BF16, tag=f"et{s}",
                             name=f"et{s}_{pair}_{qc}_{i}")
                nc.scalar.activation(et[:], sps[:], Exp, scale=0.125)
                pend[s] = (s, i, et, ups[s], pair)

            # normalize the previous block once its last AVs are emitted
            # (they were emitted above, during this step s-loop, iff i == 0)
            if i == 0 and done_block is not None:
                normalize(*done_block)
                done_block = None
            if i == KP - 1:
                done_block = (pair, qc, ups)

            # a few units of extra PE work per step (~430 ns each)
            for _ in range(pop_budget(pair, qc)):
                if extra_q:
                    extra_q.pop(0)()

        # epilogue: last step's AVs, last block's normalize, leftovers
        for s in (0, 1):
            if pend[s] is not None:
                emit_av(pend[s])
                pend[s] = None
        if done_block is not None:
            normalize(*done_block)
        for th in extra_q:
            th()
        # remaining proj chunks: t4 = NQC-1 .. TC4-1 (inline covered 0..NQC-2)
        # All four PSUM slots are free now -- rotate them for a deeper
        # chunk pipeline in this PE-bound tail.
        n = 0
        for t4 in range(NQC - 1, TC4):
            for occ in range(OCC):
                proj_chunk(occ, t4, slot=pro_slots[n % 4])
                n += 1


def build_nc(T=2048):
    FC = 2 * CG // 128
    OCC = C // 128
    nc = bacc.Bacc("TRN2", target_bir_lowering=False, debug=False,
                   num_devices=N_CORES)
    KC = C // 128
    PCH = CG // 128
    x_t = nc.dram_tensor("x_t", [C, T], BF16, kind="ExternalInput")
    # host-permuted: w_qk[fc*128+p, kc*128+c] = W[kc*128+p, fc*128+c]
    w_qk = nc.dram_tensor("w_qk", [FC * 128, KC * 128], BF16,
                          kind="ExternalInput")
    b_qk = nc.dram_tensor("b_qk", [128, FC], F32, kind="ExternalInput")
    # host-permuted: w_v[p, kc*CG+c] = Wv[kc*128+p, c]
    w_v = nc.dram_tensor("w_v", [128, KC * CG], BF16, kind="ExternalInput")
    # host-permuted: w_p[p, fcp*C+c] = Wp[fcp*128+p, c]
    w_p = nc.dram_tensor("w_p", [128, PCH * C], BF16, kind="ExternalInput")
    b_out = nc.dram_tensor("b_out", [128, OCC], F32, kind="ExternalInput")
    out_t = nc.dram_tensor("out_t", [C, T], F32, kind="ExternalOutput")
    with tile.TileContext(nc) as tc:
        _body(tc, T, x_t.ap(), w_qk.ap(), b_qk.ap(), w_v.ap(),
              w_p.ap(), b_out.ap(), out_t.ap())
    nc.compile()
    return nc


def shard_inputs(sequences, w_attn, b_attn, w_proj, b_proj):
    """Build the 8 per-core input maps. Core index = b*2 + g."""
    sequences = np.asarray(sequences, dtype=np.float32)
    w_attn = np.asarray(w_attn, dtype=np.float32)
    b_attn = np.asarray(b_attn, dtype=np.float32)
    w_proj = np.asarray(w_proj, dtype=np.float32)
    b_proj = np.asarray(b_proj, dtype=np.float32)
    B = sequences.shape[0]
    in_maps = []
    for b in range(B):
        xt_b = np.ascontiguousarray(sequences[b].T).astype(NP_BF16)
        for g in range(2):
            qs = slice(g * CG, (g + 1) * CG)
            ks = slice(C + g * CG, C + (g + 1) * CG)
            vs = slice(2 * C + g * CG, 2 * C + (g + 1) * CG)
            KC, FC, PCH = C // 128, 2 * CG // 128, CG // 128
            wqk_cat = np.concatenate([w_attn[:, qs], w_attn[:, ks]], axis=1)
            wqk_perm = (wqk_cat.reshape(KC, 128, FC, 128)
                        .transpose(2, 1, 0, 3).reshape(FC * 128, KC * 128))
            wv_perm = (w_attn[:, vs].reshape(KC, 128, CG)
                       .transpose(1, 0, 2).reshape(128, KC * CG))
            wp_perm = (w_proj[g * CG:(g + 1) * CG, :].reshape(PCH, 128, C)
                       .transpose(1, 0, 2).reshape(128, PCH * C))
            in_maps.append({
                "x_t": xt_b,
                "w_qk": np.ascontiguousarray(wqk_perm).astype(NP_BF16),
                "b_qk": np.ascontiguousarray(
                    np.concatenate([b_attn[qs], b_attn[ks]])
                    .reshape(8, 128).T),
                "w_v": np.ascontiguousarray(wv_perm).astype(NP_BF16),
                "w_p": np.ascontiguousarray(wp_perm).astype(NP_BF16),
                # softmax rows sum to 1, so the v-bias folds into the output
                # bias: y_g = attn@(x@w_v) @ w_p + (b_v@w_p [+ b_proj on g0])
                "b_out": np.ascontiguousarray(
                    (b_attn[vs] @ w_proj[g * CG:(g + 1) * CG, :]
                     + (b_proj if g == 0 else 0.0))
                    .astype(np.float32).reshape(8, 128).T),
            })
    return in_maps


def unshard_outputs(outs, B, T):
    """outs: list of 8 [C, T] partials, core index = b*2+g."""
    y = np.empty((B, T, C), np.float32)
    for b in range(B):
        y[b] = (outs[2 * b] + outs[2 * b + 1]).T
    return y


_NC_CACHE = {}


def kernel(sequences, w_attn, b_attn, w_proj, b_proj):
    sequences = np.asarray(sequences, dtype=np.float32)
    B, T, _ = sequences.shape
    in_maps = shard_inputs(sequences, w_attn, b_attn, w_proj, b_proj)
    if T not in _NC_CACHE:
        _NC_CACHE[T] = build_nc(T)
    nc = _NC_CACHE[T]
    res = run_bass_kernel_spmd(nc, in_maps, list(range(N_CORES)))
    outs = [res.results[i]["out_t"] for i in range(N_CORES)]
    return unshard_outputs(outs, B, T)


if __name__ == "__main__":
    rng = np.random.default_rng(0)
    B, T = 4, 2048
    seq = rng.standard_normal((B, T, C), dtype=np.float32)
    wa = rng.standard_normal((C, 3 * C), dtype=np.float32) / np.sqrt(C)
    ba = np.zeros(3 * C, np.float32)
    wp = rng.standard_normal((C, C), dtype=np.float32) / np.sqrt(C)
    bp = np.zeros(C, np.float32)
    y = kernel(seq, wa, ba, wp, bp)
    print(y.shape, y.dtype)


# revision 40
# speedup vs baseline: 4.4083x; 1.3131x over previous
"""Multi-head self-attention Trainium2 Bass kernel.

Problem: y = (softmax((x@Wq)(x@Wk)^T / sqrt(hd)) (x@Wv)) @ Wp + biases
with B=4, T=2048, C=1024, H=16, hd=64.

Sharding over 8 NeuronCores: (batch b in 0..3) x (head-group g in 0..1, 8
heads each) -- tensor-parallel over heads, data-parallel over batch.  Each
core computes the attention for its batch and head group plus the partial
output projection restricted to its head group's features; the host sums
the two head-group partials per batch (the row-parallel TP reduction) and
transposes back.

Per-core pipeline (T=2048, C=1024, Cg=512), built to keep the Activation
engine (exp: 33.5M elements ~ 255us busy) saturated, since it is the hard
roofline for this op:

  stage 1 (bf16): Q^T,K^T [feat,tok] and V [tok,feat] from x tiles.
  stage 2 per head-pair: scores^T chunks [key,q] in PSUM (contract hd=64,
    two heads on disjoint PE row groups), exp via ScalarE with fused
    scale=1/8 emitting bf16 E tiles [128, 1024] (two key-chunks side by
    side -- 1024-element activations amortize the ~185ns ACT access
    overhead), then AV matmuls with stationary [V_h|ones] so one M=128
    pass accumulates U^T and the replicated softmax denominator (the
    ones columns are free: matmul cost is moving rows only).
  The whole (pair, qc, kcpair) iteration space is software-pipelined:
    AV matmuls lag one step behind exp, and the PE slack in each step is
    filled with "extra" ~2-matmul work units (pair 0: remaining V chunks
    and later Q chunks; pairs 0-2: the next pair's QK chunks; pair 3:
    early output-projection chunks), so neither PE nor ScalarE waits.
  stage 3: out^T = (Y @ w_p)^T + b_out, streamed out per chunk.

  All weights are host-permuted so each loads with a single large DMA
  (DMA dispatch is ~650ns per instruction on the sync queue regardless
  of size, so few+large transfers win).
"""

import numpy as np
import ml_dtypes

import concourse.bass as bass
import concourse.bacc as bacc
import concourse.tile as tile
from concourse import mybir
from concourse.bass_utils import run_bass_kernel_spmd

N_CORES = 8
C = 1024           # embed dim
H = 16             # total heads
HD = 64            # head dim
HPC = 8            # heads per core
CG = HPC * HD      # 512: per-core q/k/v feature width

F32 = mybir.dt.float32
BF16 = mybir.dt.bfloat16
NP_BF16 = ml_dtypes.bfloat16


def _body(tc, T, wx, b_all, out_t):
    nc = tc.nc
    # unpack views of the single packed bf16 input blob [rows, 4096]:
    #   x (1024*T elems) | w_qk permuted (1024*1024) | w_v (128*4096)
    #   | w_p (128*4096).  Packing everything into one dram tensor cuts
    #   per-call PJRT argument-dispatch overhead (~13us per arg per call).
    KBLK = 4096 // T if T < 4096 else 1
    XQ = T // 4                  # w_qk base row
    XV = XQ + 256                # w_v base row
    XP = XV + 128                # w_p base row
    x_t = [wx[i * (T // 32):(i + 1) * (T // 32), :]
           .rearrange("r (k c) -> (r k) c", c=T) for i in range(C // 128)]
    w_qk_fc = [wx[XQ + fc * 32:XQ + (fc + 1) * 32, :]
               .rearrange("r (k c) -> (r k) c", c=1024)
               for fc in range(2 * CG // 128)]
    w_v = wx[XV:XV + 128, :]
    w_p = wx[XP:XP + 128, :]
    b_qk = b_all[:, 0:2 * CG // 128]
    b_out = b_all[:, 2 * CG // 128:]
    KC = C // 128            # contraction chunks over C (8)
    FC = 2 * CG // 128       # q||k feature chunks (8)
    TC4 = T // 512           # token chunks of 512 (4)
    TC1 = T // 128           # key chunks of 128 (16)
    KP = TC1 // 2            # key chunk PAIRS (8)
    OCC = C // 128           # output channel chunks (8)
    PCH = CG // 128          # proj contraction chunks (4)
    NPAIR = HPC // 2         # head pairs (4)
    QW = 512                 # q tile width in stage 2
    NQC = T // QW            # q chunks per pair (4)
    Exp = mybir.ActivationFunctionType.Exp
    Mult = mybir.AluOpType.mult

    with (
        tc.tile_pool(name="const", bufs=1) as constp,
        tc.tile_pool(name="persist", bufs=1) as pers,
        tc.tile_pool(name="wqk", bufs=2) as wqkp,
        tc.tile_pool(name="wv", bufs=1) as wvp,
        tc.tile_pool(name="wp", bufs=1) as wpp,
        tc.tile_pool(name="e", bufs=2) as ep,
        tc.tile_pool(name="rec", bufs=2) as recp,
        tc.tile_pool(name="outp", bufs=2) as outp,
        tc.tile_pool(name="ps", bufs=1, space="PSUM") as psp,
    ):
        bqk_sb = constp.tile([128, FC], F32, tag="bqk")
        nc.sync.dma_start(bqk_sb[:], b_qk[:])
        bout_sb = constp.tile([128, OCC], F32, tag="bout")

        # x tiles [128, T] bf16, one per contraction chunk.  DMA'd in
        # token-quarters (all tiles' quarter q before quarter q+1) so the
        # first QK chunks (which only need tokens 0:512 of every tile) can
        # start ~3.5us in instead of waiting for the full 4MB of x.
        xt = [pers.tile([128, T], BF16, tag=f"xt{i}", name=f"xt{i}")
              for i in range(KC)]

        # stationary V packs for the AV matmul: per key-chunk kc, [128, 8*128]
        # bf16 where cols h*128+(0:64) hold V_h for key 128*kc + p and cols
        # h*128+(64:128) hold ones (the softmax-denominator rows, replicated
        # so one M=128 matmul yields U^T and the denominator).  memset to 1.0
        # once; V copies fill the V halves.
        v2 = [pers.tile([128, 8 * 128], BF16, tag=f"v2_{i}", name=f"v2_{i}")
              for i in range(TC1)]
        for i in range(TC1):
            nc.gpsimd.memset(v2[i][:], 1.0)

        qkt = [pers.tile([128, T], BF16, tag=f"qkt{i}", name=f"qkt{i}")
               for i in range(FC)]
        yt = [pers.tile([128, T], BF16, tag=f"yt{i}", name=f"yt{i}")
              for i in range(PCH)]

        # consolidated weight tiles (host-permuted so each is ONE dma with
        # >=2KB contiguous per-partition runs -- DMA dispatch is ~650ns per
        # instruction regardless of size, so fewer+bigger wins)
        wv_all = wvp.tile([128, KC * CG], BF16, tag="wv", name="wv_all")
        wp_all = wpp.tile([128, PCH * C], BF16, tag="wp", name="wp_all")
        wv = [wv_all[:, kc * CG:(kc + 1) * CG] for kc in range(KC)]
        wp = [wp_all[:, fcp * C:(fcp + 1) * C] for fcp in range(PCH)]

        # ---------- helper emitters ----------

        def dma_wqk(pair):
            """DMA the two w_qk feature chunks (K first: fc=4+pair, then
            Q: fc=pair) -- host-permuted layout, one DMA per fc.  The tile
            holds all KC stationary blocks: [p, kc, col]."""
            tiles = {}
            for fc in (FC // 2 + pair, pair):
                t = wqkp.tile([128, KC * 128], BF16, tag=f"wqk_{fc % 2}",
                              name=f"wqk_{fc}")
                nc.sync.dma_start(t[:], w_qk_fc[fc])
                v = t.rearrange("p (kc c) -> p kc c", c=128)
                tiles[fc] = [v[:, kc, :] for kc in range(KC)]
            return tiles

        # "extra" PE work is emitted in ~2-matmul units (~430 ns) so a unit
        # fits the per-step PE slack of the ACT-bound steady state.

        def qk_units(wts, fc, t4, slot):
            """One QK output chunk qkt[fc][:, t4*512:] as KC//2 units."""
            st = {}

            def unit(u):
                if u == 0:
                    st["ps"] = psp.tile([128, 512], F32, tag=slot, bufs=1,
                                        name=f"qkps_{fc}_{t4}")
                ps = st["ps"]
                for kc in (2 * u, 2 * u + 1):
                    nc.tensor.matmul(
                        ps[:], wts[kc][:], xt[kc][:, t4 * 512:(t4 + 1) * 512],
                        start=(kc == 0), stop=(kc == KC - 1))
                if u == KC // 2 - 1:
                    nc.vector.tensor_scalar_add(
                        qkt[fc][:, t4 * 512:(t4 + 1) * 512], ps[:],
                        bqk_sb[:, fc:fc + 1])

            return [lambda u=u: unit(u) for u in range(KC // 2)]

        def v_units(tokc, slot):
            """One V token chunk -> fp8 copy into the v2 pack, as units."""
            st = {}

            def unit(u):
                if u == 0:
                    st["ps"] = psp.tile([128, CG], F32, tag=slot, bufs=1,
                                        name=f"vps_{tokc}")
                ps = st["ps"]
                for kc in (2 * u, 2 * u + 1):
                    nc.tensor.matmul(
                        ps[:], xt[kc][:, tokc * 128:(tokc + 1) * 128],
                        wv[kc][:],
                        start=(kc == 0), stop=(kc == KC - 1))
                if u == KC // 2 - 1:
                    dst = v2[tokc].rearrange("p (h c) -> p h c", c=128)
                    nc.vector.tensor_copy(
                        dst[:, :, 0:HD],
                        ps.rearrange("p (h c) -> p h c", c=HD))

            return [lambda u=u: unit(u) for u in range(KC // 2)]

        def proj_units(occ, t4, slot):
            """One projection output chunk [128, 512] -> DMA out, as units."""
            st = {}

            def unit(u):
                if u == 0:
                    st["ps"] = psp.tile([128, 512], F32, tag=slot, bufs=1,
                                        name=f"pps_{occ}_{t4}")
                ps = st["ps"]
                for fcp in (2 * u, 2 * u + 1):
                    nc.tensor.matmul(
                        ps[:], wp[fcp][:, occ * 128:(occ + 1) * 128],
                        yt[fcp][:, t4 * 512:(t4 + 1) * 512],
                        start=(fcp == 0), stop=(fcp == PCH - 1))
                if u == PCH // 2 - 1:
                    osb = outp.tile([128, 512], F32, tag="osb",
                                    name=f"osb_{occ}_{t4}")
                    nc.vector.tensor_scalar_add(osb[:], ps[:],
                                                bout_sb[:, occ:occ + 1])
                    nc.sync.dma_start(
                        out_t[occ * 128:(occ + 1) * 128,
                              t4 * 512:(t4 + 1) * 512],
                        osb[:])

            return [lambda u=u: unit(u) for u in range(PCH // 2)]

        def v_chunk(tokc, slot):
            for u in v_units(tokc, slot):
                u()

        def proj_chunk(occ, t4, slot):
            for u in proj_units(occ, t4, slot):
                u()

        def emit_av(pend):
            """AV matmuls for one step (two key chunks), one head s."""
            s, i, et, u, pair = pend
            h = 2 * pair + s
            for half in (0, 1):
                kc = 2 * i + half
                nc.tensor.matmul(
                    u[:], v2[kc][:, h * 128:(h + 1) * 128],
                    et[:, half * QW:(half + 1) * QW],
                    start=(i == 0 and half == 0),
                    stop=(i == KP - 1 and half == 1))

        def normalize(pair, qc, ups):
            q0 = qc * QW
            for s in (0, 1):
                po = s * 64
                u = ups[s]
                rec = recp.tile([64, QW], F32, tag=f"rec{s}",
                                name=f"rec{s}_{pair}_{qc}")
                nc.vector.reciprocal(rec[:], u[64:128, :])
                nc.vector.tensor_tensor(
                    yt[pair][po:po + 64, q0:q0 + QW],
                    u[0:64, :], rec[:], op=Mult)

        # ---------- DMA emission order (startup-latency critical) ----------
        # pair-0 K/Q weights, then x tiles, then V/proj weights, biases.
        wqk_tiles = {0: dma_wqk(0)}
        for i in range(KC):
            nc.sync.dma_start(xt[i][:], x_t[i])
        nc.sync.dma_start(wv_all[:], w_v[:])
        nc.sync.dma_start(wp_all[:], w_p[:])
        nc.sync.dma_start(bout_sb[:], b_out[:])

        # ---------- prologue: pair-0 K (all t4) + Q-t0, then V chunks ----
        N_PRO_V = min(TC1, 11)
        pro_slots = ["aux0", "aux1", "ups0", "ups1"]
        pro_chunks = [(FC // 2, t4) for t4 in range(TC4)] + [(0, 0)]
        for n, (fc, t4) in enumerate(pro_chunks):
            for u in qk_units(wqk_tiles[0][fc], fc, t4,
                              slot=pro_slots[n % 4]):
                u()
        for tokc in range(N_PRO_V):
            v_chunk(tokc, slot=pro_slots[tokc % 4])

        # ---------- per-block extra PE work (unit-granular) ----------
        def queue_qk_pair(pair_next, extra_q):
            wqk_tiles[pair_next] = dma_wqk(pair_next)
            for j, fc in enumerate(
                    [pair_next] * TC4 + [FC // 2 + pair_next] * TC4):
                t4 = j % TC4
                extra_q.extend(
                    qk_units(wqk_tiles[pair_next][fc], fc, t4,
                             slot=f"aux{j % 2}"))

        def block_extra(pair, qc, extra_q):
            if pair == 0:
                if qc == 0:
                    for tokc in range(N_PRO_V, TC1):
                        extra_q.extend(v_units(tokc, slot=f"aux{tokc % 2}"))
                    # pair-0 Q chunks for the later q-blocks (Q-t_qc is
                    # needed at block (0, qc); t0 was in the prologue)
                    for j, t4 in enumerate(range(1, TC4)):
                        extra_q.extend(
                            qk_units(wqk_tiles[0][0], 0, t4,
                                     slot=f"aux{j % 2}"))
                if qc == min(1, NQC - 1) and NPAIR > 1:
                    queue_qk_pair(1, extra_q)
            elif pair < NPAIR - 1:
                if qc == 0:
                    queue_qk_pair(pair + 1, extra_q)
            else:
                # early proj chunks: yt for t4 is final once block qc=t4's
                # lagged AVs and normalize have been emitted, which happens
                # at step (3, t4+1, 0) before this block's pops run.
                t4 = qc - 1
                if 0 <= t4 < TC4:
                    for occ in range(OCC):
                        extra_q.extend(
                            proj_units(occ, t4, slot=f"aux{occ % 2}"))

        def pop_budget(pair, qc):
            if pair == 0:
                return 3 if qc == 0 else 2
            if pair == NPAIR - 1:
                return 2
            return 1

        # ---------- main software-pipelined loop ----------
        steps = [(pair, qc, i)
                 for pair in range(NPAIR)
                 for qc in range(NQC)
                 for i in range(KP)]

        extra_q = []
        pend = {0: None, 1: None}    # per-s pending AV from previous step
        done_block = None            # (pair, qc, ups) awaiting normalize
        ups = None

        for (pair, qc, i) in steps:
            if i == 0 and qc == 0 and pair > 0:
                # emission-order safety: everything queued for earlier pairs
                # (e.g. this pair's QK chunks) must be emitted before this
                # pair's scores read the tiles it writes.
                while extra_q:
                    extra_q.pop(0)()
            if i == 0:
                block_extra(pair, qc, extra_q)
                ups = {s: psp.tile([128, QW], F32, tag=f"ups{s}", bufs=1,
                                   name=f"ups{s}_{pair}_{qc}")
                       for s in (0, 1)}

            qt, kt = qkt[pair], qkt[FC // 2 + pair]
            q0 = qc * QW

            for s in (0, 1):
                # lagged AV from the previous step for this s
                if pend[s] is not None:
                    emit_av(pend[s])
                    pend[s] = None
                # scores: sps_s holds key-chunks 2i | 2i+1 side by side
                po = s * 64
                sps = psp.tile([128, 2 * QW], F32, tag=f"sps{s}", bufs=1,
                               name=f"sps{s}_{pair}_{qc}_{i}")
                for half in (0, 1):
                    kc = 2 * i + half
                    nc.tensor.matmul(
                        sps[:, half * QW:(half + 1) * QW],
                        kt[po:po + 64, kc * 128:(kc + 1) * 128],
                        qt[po:po + 64, q0:q0 + QW],
                        start=True, stop=True)
                # exp -> bf16 E tile (ACT queue)
                et = ep.tile([128, 2 * QW], BF16, tag=f"et{s}",
                             name=f"et{s}_{pair}_{qc}_{i}")
                nc.scalar.activation(et[:], sps[:], Exp, scale=0.125)
                pend[s] = (s, i, et, ups[s], pair)

            # normalize the previous block once its last AVs are emitted
            # (they were emitted above, during this step s-loop, iff i == 0)
            if i == 0 and done_block is not None:
                normalize(*done_block)
                done_block = None
            if i == KP - 1:
                done_block = (pair, qc, ups)

            # a few units of extra PE work per step (~430 ns each)
            for _ in range(pop_budget(pair, qc)):
                if extra_q:
                    extra_q.pop(0)()

        # epilogue: last step's AVs, last block's normalize, leftovers
        for s in (0, 1):
            if pend[s] is not None:
                emit_av(pend[s])
                pend[s] = None
        if done_block is not None:
            normalize(*done_block)
        for th in extra_q:
            th()
        # remaining proj chunks: t4 = NQC-1 .. TC4-1 (inline covered 0..NQC-2)
        # All four PSUM slots are free now -- rotate them for a deeper
        # chunk pipeline in this PE-bound tail.
        n = 0
        for t4 in range(NQC - 1, TC4):
            for occ in range(OCC):
                proj_chunk(occ, t4, slot=pro_slots[n % 4])
                n += 1


def build_nc(T=2048):
    FC = 2 * CG // 128
    OCC = C // 128
    nc = bacc.Bacc("TRN2", target_bir_lowering=False, debug=False,
                   num_devices=N_CORES)
    # single packed bf16 input blob: x | w_qk (permuted) | w_v (permuted)
    # | w_p (permuted); see _body for the flat layout.  One f32 tensor for
    # both biases.  3 args per call instead of 8 cuts PJRT dispatch cost.
    wx = nc.dram_tensor("wx", [T // 4 + 512, 4096], BF16,
                        kind="ExternalInput")
    b_all = nc.dram_tensor("b_all", [128, FC + OCC], F32,
                           kind="ExternalInput")
    out_t = nc.dram_tensor("out_t", [C, T], F32, kind="ExternalOutput")
    with tile.TileContext(nc) as tc:
        _body(tc, T, wx.ap(), b_all.ap(), out_t.ap())
    nc.compile()
    return nc


def shard_inputs(sequences, w_attn, b_attn, w_proj, b_proj):
    """Build the 8 per-core input maps. Core index = b*2 + g."""
    sequences = np.asarray(sequences, dtype=np.float32)
    w_attn = np.asarray(w_attn, dtype=np.float32)
    b_attn = np.asarray(b_attn, dtype=np.float32)
    w_proj = np.asarray(w_proj, dtype=np.float32)
    b_proj = np.asarray(b_proj, dtype=np.float32)
    B = sequences.shape[0]
    in_maps = []
    for b in range(B):
        xt_b = np.ascontiguousarray(sequences[b].T).astype(NP_BF16)
        for g in range(2):
            qs = slice(g * CG, (g + 1) * CG)
            ks = slice(C + g * CG, C + (g + 1) * CG)
            vs = slice(2 * C + g * CG, 2 * C + (g + 1) * CG)
            KC, FC, PCH = C // 128, 2 * CG // 128, CG // 128
            T = sequences.shape[1]
            wqk_cat = np.concatenate([w_attn[:, qs], w_attn[:, ks]], axis=1)
            wqk_perm = (wqk_cat.reshape(KC, 128, FC, 128)
                        .transpose(2, 1, 0, 3).reshape(FC * 128, KC * 128))
            wv_perm = (w_attn[:, vs].reshape(KC, 128, CG)
                       .transpose(1, 0, 2).reshape(128, KC * CG))
            wp_perm = (w_proj[g * CG:(g + 1) * CG, :].reshape(PCH, 128, C)
                       .transpose(1, 0, 2).reshape(128, PCH * C))
            wx = np.concatenate([
                np.asarray(xt_b).reshape(-1),
                wqk_perm.astype(NP_BF16).reshape(-1),
                wv_perm.astype(NP_BF16).reshape(-1),
                wp_perm.astype(NP_BF16).reshape(-1),
            ]).reshape(T // 4 + 512, 4096)
            b_qk = (np.concatenate([b_attn[qs], b_attn[ks]])
                    .reshape(8, 128).T)
            # softmax rows sum to 1, so the v-bias folds into the output
            # bias: y_g = attn@(x@w_v) @ w_p + (b_v@w_p [+ b_proj on g0])
            b_out = ((b_attn[vs] @ w_proj[g * CG:(g + 1) * CG, :]
                      + (b_proj if g == 0 else 0.0))
                     .astype(np.float32).reshape(8, 128).T)
            in_maps.append({
                "wx": np.ascontiguousarray(wx),
                "b_all": np.ascontiguousarray(
                    np.concatenate([b_qk, b_out], axis=1).astype(np.float32)),
            })
    return in_maps


def unshard_outputs(outs, B, T):
    """outs: list of 8 [C, T] partials, core index = b*2+g."""
    y = np.empty((B, T, C), np.float32)
    for b in range(B):
        y[b] = (outs[2 * b] + outs[2 * b + 1]).T
    return y


_NC_CACHE = {}


def kernel(sequences, w_attn, b_attn, w_proj, b_proj):
    sequences = np.asarray(sequences, dtype=np.float32)
    B, T, _ = sequences.shape
    in_maps = shard_inputs(sequences, w_attn, b_attn, w_proj, b_proj)
    if T not in _NC_CACHE:
        _NC_CACHE[T] = build_nc(T)
    nc = _NC_CACHE[T]
    res = run_bass_kernel_spmd(nc, in_maps, list(range(N_CORES)))
    outs = [res.results[i]["out_t"] for i in range(N_CORES)]
    return unshard_outputs(outs, B, T)


if __name__ == "__main__":
    rng = np.random.default_rng(0)
    B, T = 4, 2048
    seq = rng.standard_normal((B, T, C), dtype=np.float32)
    wa = rng.standard_normal((C, 3 * C), dtype=np.float32) / np.sqrt(C)
    ba = np.zeros(3 * C, np.float32)
    wp = rng.standard_normal((C, C), dtype=np.float32) / np.sqrt(C)
    bp = np.zeros(C, np.float32)
    y = kernel(seq, wa, ba, wp, bp)
    print(y.shape, y.dtype)
